# revision 1
# baseline (speedup 1.0000x reference)
"""Trainium2 Bass kernel for nn_AnalyticalDecoder.

Evaluates 1024 2-D Gaussians (BS=16 x T=64) on a fixed 128x128 grid and
min/max-normalizes each Gaussian's field.  Output [16,64,1,128,128] f32.

Strategy (data-parallel over the 8 NeuronCores, 128 Gaussians per core,
one Gaussian per SBUF partition):
  * s(g, j, i) = -0.5 * (p - mu)^T Sigma^-1 (p - mu) is a 2-D quadratic in
    the grid coords, so the whole 16384-point field per Gaussian is a K=6
    matmul against a constant polynomial basis {xi^2, xi*xj, xj^2, xi, xj, 1}
    (TensorE), exp on ScalarE, and an affine normalize on VectorE.
  * The normalization prefactor exp(-log2pi - 0.5 log det) cancels in
    (p - mn) / (mx - mn), so only s itself is needed.
  * min over the grid of a concave quadratic is attained exactly at one of
    the 4 grid corners.  max is found per-row in closed form: for each y_j
    the 1-D restriction is a concave parabola whose discrete argmax is the
    grid point nearest its vertex; a 128-point reduce over rows finishes it.
    This avoids two full 16K-element reduction passes per partition.
  * out = exp(s - smax) * r1 - r2 with r1 = 1/(1 - exp(smin-smax)),
    r2 = exp(smin-smax) * r1 -- exactly (e^s - e^smin)/(e^smax - e^smin).
"""

import numpy as np

import concourse.bass as bass
import concourse.bacc as bacc
import concourse.tile as tile
from concourse import mybir
from concourse.bass_utils import run_bass_kernel_spmd

RES = 128
NPTS = RES * RES          # 16384
N_CORES = 8
G_PER_CORE = 128          # 16*64 / 8
H = 30.0 / 127.0          # grid spacing
MAGIC = 12582912.0        # 1.5 * 2**23: (x + MAGIC) - MAGIC == round(x) for |x| < 2**22

CHUNK = 2048              # ACT/DVE/DMA chunk = 4 PSUM banks
N_CHUNKS = NPTS // CHUNK  # 8
MM_N = 512                # matmul free dim = 1 PSUM bank
MM_PER_CHUNK = CHUNK // MM_N

MM_DTYPE = mybir.dt.float32   # float32 = exact (4 cyc/row); float32r = fast (1 cyc/row)


def build_nc():
    nc = bacc.Bacc("TRN2", target_bir_lowering=False, debug=False)
    f32 = mybir.dt.float32
    AOp = mybir.AluOpType
    FT = mybir.ActivationFunctionType
    X = mybir.AxisListType.X

    params_d = nc.dram_tensor("params", [G_PER_CORE, 8], f32, kind="ExternalInput")
    basis_d = nc.dram_tensor("basis", [6, NPTS], MM_DTYPE, kind="ExternalInput")
    ygrid_d = nc.dram_tensor("ygrid", [G_PER_CORE, RES], f32, kind="ExternalInput")
    corners_d = nc.dram_tensor("corners", [G_PER_CORE, 8], f32, kind="ExternalInput")
    ident_d = nc.dram_tensor("ident", [128, 128], f32, kind="ExternalInput")
    out_d = nc.dram_tensor("out", [G_PER_CORE, NPTS], f32, kind="ExternalOutput")
    out_ap = out_d.ap()

    with tile.TileContext(nc) as tc:
        with (
            tc.tile_pool(name="const", bufs=1) as cpool,
            tc.tile_pool(name="small", bufs=1) as sp,
            tc.tile_pool(name="mid", bufs=1) as mp,
            tc.tile_pool(name="psum", bufs=2, space=bass.MemorySpace.PSUM) as pp,
            tc.tile_pool(name="io", bufs=3) as iop,
        ):
            P = cpool.tile([128, 8], f32)
            nc.sync.dma_start(P[:], params_d.ap())
            YG = cpool.tile([128, RES], f32)
            nc.sync.dma_start(YG[:], ygrid_d.ap())
            CRt = cpool.tile([128, 8], f32)
            nc.sync.dma_start(CRt[:], corners_d.ap())
            ID = cpool.tile([128, 128], f32)
            nc.sync.dma_start(ID[:], ident_d.ap())
            BSt = cpool.tile([6, NPTS], MM_DTYPE)
            nc.sync.dma_start(BSt[:], basis_d.ap())

            mux = P[:, 0:1]
            muy = P[:, 1:2]
            a = P[:, 2:3]
            b = P[:, 3:4]
            c = P[:, 4:5]
            d = P[:, 5:6]

            # Basis coefficients per Gaussian, assembled in [128, 6] for transpose.
            # rows of lhsT: A (xi^2), B (xi*xj), C (xj^2), cxi (xi), cxj (xj), c0 - smax (1)
            CF = sp.tile([128, 6], f32)
            A_ = CF[:, 0:1]
            B_ = CF[:, 1:2]
            C_ = CF[:, 2:3]

            t_ad = sp.tile([128, 1], f32)
            nc.vector.tensor_mul(t_ad[:], a, d)
            ndet = sp.tile([128, 1], f32)  # b*c - a*d = -det
            nc.vector.scalar_tensor_tensor(ndet[:], b, c, t_ad[:], AOp.mult, AOp.subtract)
            nhalf = sp.tile([128, 1], f32)  # -1/det
            nc.vector.reciprocal(nhalf[:], ndet[:])
            nhi = sp.tile([128, 1], f32)  # -0.5/det
            nc.vector.tensor_scalar_mul(nhi[:], nhalf[:], 0.5)
            nc.vector.tensor_mul(A_, d, nhi[:])          # A = -0.5*d/det
            nc.vector.tensor_mul(C_, a, nhi[:])          # C = -0.5*a/det
            bsum = sp.tile([128, 1], f32)
            nc.vector.tensor_add(bsum[:], b, c)
            nc.vector.scalar_tensor_tensor(B_, bsum[:], -1.0, nhi[:], AOp.mult, AOp.mult)  # B = 0.5*(b+c)/det

            t1 = sp.tile([128, 1], f32)  # -2*A*mux
            nc.vector.scalar_tensor_tensor(t1[:], mux, -2.0, A_, AOp.mult, AOp.mult)
            t2 = sp.tile([128, 1], f32)
            nc.vector.tensor_mul(t2[:], B_, muy)
            nc.vector.tensor_sub(CF[:, 3:4], t1[:], t2[:])   # cxi = -2A*mux - B*muy

            t3 = sp.tile([128, 1], f32)  # -2*C*muy
            nc.vector.scalar_tensor_tensor(t3[:], muy, -2.0, C_, AOp.mult, AOp.mult)
            t4 = sp.tile([128, 1], f32)
            nc.vector.tensor_mul(t4[:], B_, mux)
            nc.vector.tensor_sub(CF[:, 4:5], t3[:], t4[:])   # cxj = -2C*muy - B*mux

            m1 = sp.tile([128, 1], f32)
            nc.vector.scalar_tensor_tensor(m1[:], mux, A_, mux, AOp.mult, AOp.mult)  # A*mux^2
            m2 = sp.tile([128, 1], f32)
            nc.vector.scalar_tensor_tensor(m2[:], muy, C_, muy, AOp.mult, AOp.mult)  # C*muy^2
            m3 = sp.tile([128, 1], f32)
            nc.vector.scalar_tensor_tensor(m3[:], mux, B_, muy, AOp.mult, AOp.mult)  # B*mux*muy
            c0 = sp.tile([128, 1], f32)
            nc.vector.tensor_add(c0[:], m1[:], m2[:])
            nc.vector.tensor_add(c0[:], c0[:], m3[:])

            rA = sp.tile([128, 1], f32)
            nc.vector.reciprocal(rA[:], A_)
            kfh = sp.tile([128, 1], f32)   # -B/(2A)/H
            nc.vector.scalar_tensor_tensor(kfh[:], B_, -0.5 / H, rA[:], AOp.mult, AOp.mult)
            mxh = sp.tile([128, 1], f32)   # (mux+15)/H
            nc.vector.tensor_scalar(mxh[:], mux, 15.0, 1.0 / H, AOp.add, AOp.mult)
            nmx15 = sp.tile([128, 1], f32)  # -(mux+15)
            nc.vector.tensor_scalar(nmx15[:], mux, 15.0, -1.0, AOp.add, AOp.mult)

            # --- smax: per-row closed-form argmax, then 128-point reduce ---
            dy = mp.tile([128, RES], f32)
            nc.vector.tensor_scalar_sub(dy[:], YG[:], muy)
            tq = mp.tile([128, RES], f32)   # continuous col index of row-argmax
            nc.vector.tensor_scalar(tq[:], dy[:], kfh[:], mxh[:], AOp.mult, AOp.add)
            tqc = mp.tile([128, RES], f32)
            nc.vector.tensor_scalar(tqc[:], tq[:], 0.0, 127.0, AOp.max, AOp.min)
            tqr = mp.tile([128, RES], f32)  # round to nearest grid index
            nc.vector.tensor_scalar(tqr[:], tqc[:], MAGIC, MAGIC, AOp.add, AOp.subtract)
            dxq = mp.tile([128, RES], f32)  # x_q - mux
            nc.vector.tensor_scalar(dxq[:], tqr[:], H, nmx15[:], AOp.mult, AOp.add)
            w1 = mp.tile([128, RES], f32)
            nc.vector.tensor_scalar_mul(w1[:], dy[:], C_)
            w2 = mp.tile([128, RES], f32)
            nc.vector.scalar_tensor_tensor(w2[:], dxq[:], B_, w1[:], AOp.mult, AOp.add)
            w3 = mp.tile([128, RES], f32)
            nc.vector.tensor_mul(w3[:], w2[:], dy[:])
            w4 = mp.tile([128, RES], f32)
            nc.vector.scalar_tensor_tensor(w4[:], dxq[:], A_, dxq[:], AOp.mult, AOp.mult)
            mrow = mp.tile([128, RES], f32)
            nc.vector.tensor_add(mrow[:], w3[:], w4[:])
            smax = sp.tile([128, 1], f32)
            nc.vector.tensor_reduce(smax[:], mrow[:], X, AOp.max)

            # --- smin: exact at one of the 4 grid corners (s is concave) ---
            dxc = sp.tile([128, 4], f32)
            nc.vector.tensor_scalar_sub(dxc[:], CRt[:, 0:4], mux)
            dyc = sp.tile([128, 4], f32)
            nc.vector.tensor_scalar_sub(dyc[:], CRt[:, 4:8], muy)
            z1 = sp.tile([128, 4], f32)
            nc.vector.tensor_scalar_mul(z1[:], dyc[:], C_)
            z2 = sp.tile([128, 4], f32)
            nc.vector.scalar_tensor_tensor(z2[:], dxc[:], B_, z1[:], AOp.mult, AOp.add)
            z3 = sp.tile([128, 4], f32)
            nc.vector.tensor_mul(z3[:], z2[:], dyc[:])
            z4 = sp.tile([128, 4], f32)
            nc.vector.scalar_tensor_tensor(z4[:], dxc[:], A_, dxc[:], AOp.mult, AOp.mult)
            zm = sp.tile([128, 4], f32)
            nc.vector.tensor_add(zm[:], z3[:], z4[:])
            smin = sp.tile([128, 1], f32)
            nc.vector.tensor_reduce(smin[:], zm[:], X, AOp.min)

            # --- normalization scalars ---
            tdiff = sp.tile([128, 1], f32)
            nc.vector.tensor_sub(tdiff[:], smin[:], smax[:])
            et = sp.tile([128, 1], f32)
            nc.scalar.activation(et[:], tdiff[:], FT.Exp)
            om = sp.tile([128, 1], f32)   # 1 - et
            nc.vector.tensor_scalar(om[:], et[:], -1.0, 1.0, AOp.mult, AOp.add)
            r1 = sp.tile([128, 1], f32)
            nc.vector.reciprocal(r1[:], om[:])
            r2 = sp.tile([128, 1], f32)
            nc.vector.tensor_mul(r2[:], et[:], r1[:])
            nc.vector.tensor_sub(CF[:, 5:6], c0[:], smax[:])

            # --- transpose coefficients [128,6] -> lhsT [6,128] via PE ---
            cfT_ps = pp.tile([6, 128], f32, tag="ps")
            nc.tensor.transpose(cfT_ps[:], CF[:, 0:6], ID[:])
            lhsT = cpool.tile([6, 128], MM_DTYPE)
            nc.vector.tensor_copy(lhsT[:], cfT_ps[:])

            # --- main loop: matmul -> exp -> normalize -> DMA out ---
            for ch in range(N_CHUNKS):
                ps = pp.tile([128, CHUNK], f32, tag="ps")
                for mm in range(MM_PER_CHUNK):
                    lo = ch * CHUNK + mm * MM_N
                    nc.tensor.matmul(
                        ps[:, mm * MM_N:(mm + 1) * MM_N],
                        lhsT[:],
                        BSt[:, lo:lo + MM_N],
                        start=True,
                        stop=True,
                    )
                e = iop.tile([128, CHUNK], f32, tag="e")
                nc.scalar.activation(e[:], ps[:], FT.Exp)
                o = iop.tile([128, CHUNK], f32, tag="o")
                nc.vector.tensor_scalar(o[:], e[:], r1[:], r2[:], AOp.mult, AOp.subtract)
                nc.sync.dma_start(out_ap[:, ch * CHUNK:(ch + 1) * CHUNK], o[:])

    nc.compile()
    return nc


def make_constants():
    x64 = np.linspace(-15.0, 15.0, RES)          # float64, like the reference
    xf = x64.astype(np.float32)
    xi = np.tile(x64, RES)                       # flat idx n = j*RES + i -> x[i]
    xj = np.repeat(x64, RES)                     # -> x[j]
    basis = np.stack(
        [xi * xi, xi * xj, xj * xj, xi, xj, np.ones(NPTS)]
    ).astype(np.float32)
    ygrid = np.tile(xf, (G_PER_CORE, 1))
    corners = np.zeros((G_PER_CORE, 8), np.float32)
    corners[:, 0:4] = np.array([-15.0, 15.0, -15.0, 15.0], np.float32)
    corners[:, 4:8] = np.array([-15.0, -15.0, 15.0, 15.0], np.float32)
    ident = np.eye(128, dtype=np.float32)
    return basis, ygrid, corners, ident


def make_in_maps(mu, covar):
    mu = np.ascontiguousarray(np.asarray(mu), dtype=np.float32)
    covar = np.ascontiguousarray(np.asarray(covar), dtype=np.float32)
    G = mu.shape[0] * mu.shape[1]
    muf = mu.reshape(G, 2)
    cvf = covar.reshape(G, 4)
    basis, ygrid, corners, ident = make_constants()
    in_maps = []
    for cid in range(N_CORES):
        sl = slice(cid * G_PER_CORE, (cid + 1) * G_PER_CORE)
        params = np.zeros((G_PER_CORE, 8), np.float32)
        params[:, 0] = muf[sl, 0]
        params[:, 1] = muf[sl, 1]
        params[:, 2] = cvf[sl, 0]   # a
        params[:, 3] = cvf[sl, 1]   # b
        params[:, 4] = cvf[sl, 2]   # c
        params[:, 5] = cvf[sl, 3]   # d
        in_maps.append(
            {
                "params": params,
                "basis": basis,
                "ygrid": ygrid,
                "corners": corners,
                "ident": ident,
            }
        )
    return in_maps


_NC_CACHE = None


def get_nc():
    global _NC_CACHE
    if _NC_CACHE is None:
        _NC_CACHE = build_nc()
    return _NC_CACHE


def kernel(mu, covar, _trace=False, _trace_kwargs=None):
    in_maps = make_in_maps(mu, covar)
    nc = get_nc()
    res = run_bass_kernel_spmd(
        nc, in_maps, core_ids=list(range(N_CORES)), trace=_trace,
        **(_trace_kwargs or {}),
    )
    outs = [np.asarray(res.results[i]["out"]) for i in range(N_CORES)]
    full = np.concatenate(outs, axis=0)           # [1024, 16384]
    out = full.reshape(16, 64, 1, RES, RES).astype(np.float32, copy=False)
    if _trace:
        return out, res
    return out


# revision 3
# speedup vs baseline: 1.7175x; 1.7175x over previous
"""Trainium2 Bass kernel for nn_AnalyticalDecoder.

Evaluates 1024 2-D Gaussians (BS=16 x T=64) on a fixed 128x128 grid and
min/max-normalizes each Gaussian's field.  Output [16,64,1,128,128] f32.

Strategy (data-parallel over the 8 NeuronCores, 128 Gaussians per core,
one Gaussian per SBUF partition):
  * Work in grid-index coordinates (i, j in 0..127).  s(g, j, i) =
    -0.5 (p-mu)^T Sigma^-1 (p-mu) = Ai*di^2 + Bi*di*dj + Ci*dj^2 (di = i-mi)
    is a quadratic in (i, j), so the 16384-point field per Gaussian is a
    matmul against a constant basis {i^2, i*j, j^2, i, j} (TensorE), exp on
    ScalarE (constant term rides the exp bias), affine normalize on VectorE.
  * Precision trick: fp32 matmul is 4-5 cyc/row and fp32r loses ~12 mantissa
    bits (fatal in the monomial cancellation near the peak).  Instead each
    integer basis product is split exactly as v = 128*q + r with q, r < 128
    -- exactly representable in bf16 -- and each of the 8 coefficients is
    split into hi/mid/lo bf16 parts (24 mantissa bits).  The K=24 bf16
    matmul streams at 1 cyc/row (K is free on the 128-deep PE), with exact
    basis values and fp32-accurate coefficients.
  * The normalization prefactor exp(-log2pi - 0.5 log det) cancels in
    (p - mn)/(mx - mn), so only s is needed.
  * min over the grid of the concave quadratic s is attained exactly at one
    of the 4 grid corners.  max: for each row j the restriction to i is a
    concave parabola whose discrete argmax is the grid point nearest its
    vertex (closed form), then a 128-point reduce over rows.  This avoids
    two full 16K-element reduction passes per partition.
  * out = exp(s - smax) * r1 - r2 with r1 = 1/(1 - exp(smin-smax)),
    r2 = exp(smin-smax) * r1 -- exactly (e^s - e^smin)/(e^smax - e^smin).
"""

import ml_dtypes
import numpy as np

import concourse.bass as bass
import concourse.bacc as bacc
import concourse.tile as tile
from concourse import mybir
from concourse.bass_utils import run_bass_kernel_spmd

RES = 128
NPTS = RES * RES          # 16384
N_CORES = 8
G_PER_CORE = 128          # 16*64 / 8
H = 30.0 / 127.0          # grid spacing
MAGIC = 12582912.0        # 1.5 * 2**23: (x + MAGIC) - MAGIC == round(x) for |x| < 2**22
KB8 = 8                   # basis rows: q_ii, r_ii, q_ij, r_ij, q_jj, r_jj, i, j
KB = 3 * KB8              # hi/mid/lo coefficient splits

CHUNK = 2048              # ACT/DVE/DMA chunk = 4 PSUM banks
N_CHUNKS = NPTS // CHUNK  # 8
MM_N = 512                # matmul free dim = 1 PSUM bank
MM_PER_CHUNK = CHUNK // MM_N


def build_nc():
    nc = bacc.Bacc("TRN2", target_bir_lowering=False, debug=False)
    f32 = mybir.dt.float32
    bf16 = mybir.dt.bfloat16
    AOp = mybir.AluOpType
    FT = mybir.ActivationFunctionType
    X = mybir.AxisListType.X

    params_d = nc.dram_tensor("params", [G_PER_CORE, 8], f32, kind="ExternalInput")
    basis_d = nc.dram_tensor("basis", [KB, NPTS], bf16, kind="ExternalInput")
    jgrid_d = nc.dram_tensor("jgrid", [G_PER_CORE, RES], f32, kind="ExternalInput")
    corners_d = nc.dram_tensor("corners", [G_PER_CORE, 8], f32, kind="ExternalInput")
    ident_d = nc.dram_tensor("ident", [128, 128], bf16, kind="ExternalInput")
    out_d = nc.dram_tensor("out", [G_PER_CORE, NPTS], f32, kind="ExternalOutput")
    out_ap = out_d.ap()

    with tile.TileContext(nc) as tc:
        with (
            tc.tile_pool(name="const", bufs=1) as cpool,
            tc.tile_pool(name="small", bufs=1) as sp,
            tc.tile_pool(name="mid", bufs=1) as mp,
            tc.tile_pool(name="psum", bufs=2, space=bass.MemorySpace.PSUM) as pp,
            tc.tile_pool(name="io", bufs=3) as iop,
        ):
            P = cpool.tile([128, 8], f32)
            nc.sync.dma_start(P[:], params_d.ap())
            ID = cpool.tile([128, 128], bf16)
            nc.sync.dma_start(ID[:], ident_d.ap())
            BSt = cpool.tile([KB, NPTS], bf16)
            nc.sync.dma_start(BSt[:], basis_d.ap())
            JG = cpool.tile([128, RES], f32)
            nc.sync.dma_start(JG[:], jgrid_d.ap())
            CRt = cpool.tile([128, 8], f32)
            nc.sync.dma_start(CRt[:], corners_d.ap())

            mux = P[:, 0:1]
            muy = P[:, 1:2]
            a = P[:, 2:3]
            b = P[:, 3:4]
            c = P[:, 4:5]
            d = P[:, 5:6]

            # Index-space quadratic coefficients per Gaussian, in [128, 8]
            # matching the basis rows: 128Ai, Ai, 128Bi, Bi, 128Ci, Ci, dlt, eps
            CF = sp.tile([128, KB8], f32)
            Ai = CF[:, 1:2]
            Bi = CF[:, 3:4]
            Ci = CF[:, 5:6]

            t_ad = sp.tile([128, 1], f32)
            nc.vector.tensor_mul(t_ad[:], a, d)
            ndet = sp.tile([128, 1], f32)  # b*c - a*d = -det
            nc.vector.scalar_tensor_tensor(ndet[:], b, c, t_ad[:], AOp.mult, AOp.subtract)
            nhalf = sp.tile([128, 1], f32)  # -1/det
            nc.vector.reciprocal(nhalf[:], ndet[:])
            nhi = sp.tile([128, 1], f32)  # -0.5*h^2/det
            nc.vector.tensor_scalar_mul(nhi[:], nhalf[:], 0.5 * H * H)
            nc.vector.tensor_mul(Ai, d, nhi[:])          # Ai = -0.5*h^2*d/det
            nc.vector.tensor_mul(Ci, a, nhi[:])          # Ci = -0.5*h^2*a/det
            bsum = sp.tile([128, 1], f32)
            nc.vector.tensor_add(bsum[:], b, c)
            nc.vector.scalar_tensor_tensor(Bi, bsum[:], -1.0, nhi[:], AOp.mult, AOp.mult)
            nc.vector.tensor_scalar_mul(CF[:, 0:1], Ai, 128.0)
            nc.vector.tensor_scalar_mul(CF[:, 2:3], Bi, 128.0)
            nc.vector.tensor_scalar_mul(CF[:, 4:5], Ci, 128.0)

            mi = sp.tile([128, 1], f32)   # (mux+15)/h: continuous index of mux
            nc.vector.tensor_scalar(mi[:], mux, 15.0, 1.0 / H, AOp.add, AOp.mult)
            mj = sp.tile([128, 1], f32)
            nc.vector.tensor_scalar(mj[:], muy, 15.0, 1.0 / H, AOp.add, AOp.mult)

            t1 = sp.tile([128, 1], f32)  # -2*Ai*mi
            nc.vector.scalar_tensor_tensor(t1[:], mi[:], -2.0, Ai, AOp.mult, AOp.mult)
            t2 = sp.tile([128, 1], f32)
            nc.vector.tensor_mul(t2[:], Bi, mj[:])
            nc.vector.tensor_sub(CF[:, 6:7], t1[:], t2[:])   # dlt = -2Ai*mi - Bi*mj

            t3 = sp.tile([128, 1], f32)  # -2*Ci*mj
            nc.vector.scalar_tensor_tensor(t3[:], mj[:], -2.0, Ci, AOp.mult, AOp.mult)
            t4 = sp.tile([128, 1], f32)
            nc.vector.tensor_mul(t4[:], Bi, mi[:])
            nc.vector.tensor_sub(CF[:, 7:8], t3[:], t4[:])   # eps = -2Ci*mj - Bi*mi

            # --- split coefficients into hi/mid/lo bf16 (exact to fp32) ---
            CF24 = sp.tile([128, KB], bf16)
            nc.vector.tensor_copy(CF24[:, 0:KB8], CF[:])
            res1 = sp.tile([128, KB8], f32)
            nc.vector.tensor_sub(res1[:], CF[:], CF24[:, 0:KB8])
            nc.vector.tensor_copy(CF24[:, KB8:2 * KB8], res1[:])
            res2 = sp.tile([128, KB8], f32)
            nc.vector.tensor_sub(res2[:], res1[:], CF24[:, KB8:2 * KB8])
            nc.vector.tensor_copy(CF24[:, 2 * KB8:KB], res2[:])

            # --- transpose [128,24] -> lhsT [24,128] via PE ---
            cfT_ps = pp.tile([KB, 128], bf16, tag="ps")
            nc.tensor.transpose(cfT_ps[:], CF24[:], ID[:])
            lhsT = cpool.tile([KB, 128], bf16)
            nc.vector.tensor_copy(lhsT[:], cfT_ps[:])

            # --- everything below only gates the exp/normalize stages ---
            m1 = sp.tile([128, 1], f32)
            nc.vector.scalar_tensor_tensor(m1[:], mi[:], Ai, mi[:], AOp.mult, AOp.mult)
            m2 = sp.tile([128, 1], f32)
            nc.vector.scalar_tensor_tensor(m2[:], mj[:], Ci, mj[:], AOp.mult, AOp.mult)
            m3 = sp.tile([128, 1], f32)
            nc.vector.scalar_tensor_tensor(m3[:], mi[:], Bi, mj[:], AOp.mult, AOp.mult)
            c0 = sp.tile([128, 1], f32)   # Ai*mi^2 + Bi*mi*mj + Ci*mj^2
            nc.vector.tensor_add(c0[:], m1[:], m2[:])
            nc.vector.tensor_add(c0[:], c0[:], m3[:])

            rA = sp.tile([128, 1], f32)
            nc.vector.reciprocal(rA[:], Ai)
            kf = sp.tile([128, 1], f32)   # -Bi/(2Ai)
            nc.vector.scalar_tensor_tensor(kf[:], Bi, -0.5, rA[:], AOp.mult, AOp.mult)

            # smax: per-row closed-form argmax, then 128-point reduce
            dyj = mp.tile([128, RES], f32)
            nc.vector.tensor_scalar_sub(dyj[:], JG[:], mj[:])
            tq = mp.tile([128, RES], f32)   # continuous col index of row-argmax
            nc.vector.tensor_scalar(tq[:], dyj[:], kf[:], mi[:], AOp.mult, AOp.add)
            tqc = mp.tile([128, RES], f32)
            nc.vector.tensor_scalar(tqc[:], tq[:], 0.0, 127.0, AOp.max, AOp.min)
            tqr = mp.tile([128, RES], f32)  # round to nearest grid index
            nc.vector.tensor_scalar(tqr[:], tqc[:], MAGIC, MAGIC, AOp.add, AOp.subtract)
            dqi = mp.tile([128, RES], f32)  # i_q - mi
            nc.vector.tensor_scalar_sub(dqi[:], tqr[:], mi[:])
            w1 = mp.tile([128, RES], f32)
            nc.vector.tensor_scalar_mul(w1[:], dyj[:], Ci)
            w2 = mp.tile([128, RES], f32)
            nc.vector.scalar_tensor_tensor(w2[:], dqi[:], Bi, w1[:], AOp.mult, AOp.add)
            w3 = mp.tile([128, RES], f32)
            nc.vector.tensor_mul(w3[:], w2[:], dyj[:])
            w4 = mp.tile([128, RES], f32)
            nc.vector.scalar_tensor_tensor(w4[:], dqi[:], Ai, dqi[:], AOp.mult, AOp.mult)
            mrow = mp.tile([128, RES], f32)
            nc.vector.tensor_add(mrow[:], w3[:], w4[:])
            smax = sp.tile([128, 1], f32)
            nc.vector.tensor_reduce(smax[:], mrow[:], X, AOp.max)

            # smin: exact at one of the 4 grid corners (s is concave)
            dxc = sp.tile([128, 4], f32)
            nc.vector.tensor_scalar_sub(dxc[:], CRt[:, 0:4], mi[:])
            dyc = sp.tile([128, 4], f32)
            nc.vector.tensor_scalar_sub(dyc[:], CRt[:, 4:8], mj[:])
            z1 = sp.tile([128, 4], f32)
            nc.vector.tensor_scalar_mul(z1[:], dyc[:], Ci)
            z2 = sp.tile([128, 4], f32)
            nc.vector.scalar_tensor_tensor(z2[:], dxc[:], Bi, z1[:], AOp.mult, AOp.add)
            z3 = sp.tile([128, 4], f32)
            nc.vector.tensor_mul(z3[:], z2[:], dyc[:])
            z4 = sp.tile([128, 4], f32)
            nc.vector.scalar_tensor_tensor(z4[:], dxc[:], Ai, dxc[:], AOp.mult, AOp.mult)
            zm = sp.tile([128, 4], f32)
            nc.vector.tensor_add(zm[:], z3[:], z4[:])
            smin = sp.tile([128, 1], f32)
            nc.vector.tensor_reduce(smin[:], zm[:], X, AOp.min)

            # normalization scalars
            ebias = sp.tile([128, 1], f32)   # c0 - smax  (exp input bias)
            nc.vector.tensor_sub(ebias[:], c0[:], smax[:])
            tdiff = sp.tile([128, 1], f32)
            nc.vector.tensor_sub(tdiff[:], smin[:], smax[:])
            et = sp.tile([128, 1], f32)
            nc.scalar.activation(et[:], tdiff[:], FT.Exp)
            om = sp.tile([128, 1], f32)   # 1 - et
            nc.vector.tensor_scalar(om[:], et[:], -1.0, 1.0, AOp.mult, AOp.add)
            r1 = sp.tile([128, 1], f32)
            nc.vector.reciprocal(r1[:], om[:])
            r2 = sp.tile([128, 1], f32)
            nc.vector.tensor_mul(r2[:], et[:], r1[:])

            # --- main loop: matmul -> exp(+bias) -> normalize -> DMA out ---
            for ch in range(N_CHUNKS):
                ps = pp.tile([128, CHUNK], f32, tag="ps")
                for mm in range(MM_PER_CHUNK):
                    lo = ch * CHUNK + mm * MM_N
                    nc.tensor.matmul(
                        ps[:, mm * MM_N:(mm + 1) * MM_N],
                        lhsT[:],
                        BSt[:, lo:lo + MM_N],
                        start=True,
                        stop=True,
                    )
                e = iop.tile([128, CHUNK], f32, tag="e")
                nc.scalar.activation(e[:], ps[:], FT.Exp, bias=ebias[:])
                o = iop.tile([128, CHUNK], f32, tag="o")
                nc.vector.tensor_scalar(o[:], e[:], r1[:], r2[:], AOp.mult, AOp.subtract)
                nc.sync.dma_start(out_ap[:, ch * CHUNK:(ch + 1) * CHUNK], o[:])

    nc.compile()
    return nc


def make_constants():
    idx = np.arange(RES, dtype=np.int64)
    i = np.tile(idx, RES)                        # flat idx n = j*RES + i
    j = np.repeat(idx, RES)
    rows8 = []
    for prod in (i * i, i * j, j * j):
        rows8.append(prod // 128)                # q < 128
        rows8.append(prod % 128)                 # r < 128
    rows8.append(i)
    rows8.append(j)
    basis8 = np.stack(rows8).astype(np.float64)  # all small ints, exact in bf16
    basis = np.concatenate([basis8, basis8, basis8]).astype(ml_dtypes.bfloat16)
    jgrid = np.tile(idx.astype(np.float32), (G_PER_CORE, 1))
    corners = np.zeros((G_PER_CORE, 8), np.float32)
    corners[:, 0:4] = np.array([0.0, 127.0, 0.0, 127.0], np.float32)
    corners[:, 4:8] = np.array([0.0, 0.0, 127.0, 127.0], np.float32)
    ident = np.eye(128, dtype=ml_dtypes.bfloat16)
    return basis, jgrid, corners, ident


def make_in_maps(mu, covar):
    mu = np.ascontiguousarray(np.asarray(mu), dtype=np.float32)
    covar = np.ascontiguousarray(np.asarray(covar), dtype=np.float32)
    G = mu.shape[0] * mu.shape[1]
    muf = mu.reshape(G, 2)
    cvf = covar.reshape(G, 4)
    basis, jgrid, corners, ident = make_constants()
    in_maps = []
    for cid in range(N_CORES):
        sl = slice(cid * G_PER_CORE, (cid + 1) * G_PER_CORE)
        params = np.zeros((G_PER_CORE, 8), np.float32)
        params[:, 0] = muf[sl, 0]
        params[:, 1] = muf[sl, 1]
        params[:, 2] = cvf[sl, 0]   # a
        params[:, 3] = cvf[sl, 1]   # b
        params[:, 4] = cvf[sl, 2]   # c
        params[:, 5] = cvf[sl, 3]   # d
        in_maps.append(
            {
                "params": params,
                "basis": basis,
                "jgrid": jgrid,
                "corners": corners,
                "ident": ident,
            }
        )
    return in_maps


_NC_CACHE = None


def get_nc():
    global _NC_CACHE
    if _NC_CACHE is None:
        _NC_CACHE = build_nc()
    return _NC_CACHE


def kernel(mu, covar, _trace=False, _trace_kwargs=None):
    in_maps = make_in_maps(mu, covar)
    nc = get_nc()
    res = run_bass_kernel_spmd(
        nc, in_maps, core_ids=list(range(N_CORES)), trace=_trace,
        **(_trace_kwargs or {}),
    )
    outs = [np.asarray(res.results[i]["out"]) for i in range(N_CORES)]
    full = np.concatenate(outs, axis=0)           # [1024, 16384]
    out = full.reshape(16, 64, 1, RES, RES).astype(np.float32, copy=False)
    if _trace:
        return out, res
    return out


# revision 9
# speedup vs baseline: 1.7193x; 1.0011x over previous
"""Trainium2 Bass kernel for nn_AnalyticalDecoder.

Evaluates 1024 2-D Gaussians (BS=16 x T=64) on a fixed 128x128 grid and
min/max-normalizes each Gaussian's field.  Output [16,64,1,128,128] f32.

Strategy (data-parallel over the 8 NeuronCores, 128 Gaussians per core,
one Gaussian per SBUF partition):
  * Work in grid-index coordinates (i, j in 0..127).  s(g, j, i) =
    -0.5 (p-mu)^T Sigma^-1 (p-mu) = Ai*di^2 + Bi*di*dj + Ci*dj^2 (di = i-mi)
    is a quadratic in (i, j), so the 16384-point field per Gaussian is a
    matmul against a constant basis {i^2, i*j, j^2, i, j} (TensorE), exp on
    ScalarE (constant term rides the exp bias), affine normalize on VectorE.
  * Precision trick: fp32 matmul is 4-5 cyc/row and fp32r loses ~12 mantissa
    bits (fatal in the monomial cancellation near the peak).  Instead each
    integer basis product is split exactly as v = 128*q + r with q, r < 128
    -- exactly representable in bf16 -- and each of the 8 coefficients is
    split into hi/mid/lo bf16 parts (24 mantissa bits).  The K=24 bf16
    matmul streams at 1 cyc/row (K is free on the 128-deep PE), with exact
    basis values and fp32-accurate coefficients.
  * The normalization prefactor exp(-log2pi - 0.5 log det) cancels in
    (p - mn)/(mx - mn), so only s is needed.
  * min over the grid of the concave quadratic s is attained exactly at one
    of the 4 grid corners.  max: for each row j the restriction to i is a
    concave parabola whose discrete argmax is the grid point nearest its
    vertex (closed form), then a 128-point reduce over rows.  This avoids
    two full 16K-element reduction passes per partition.
  * out = exp(s - smax) * r1 - r2 with r1 = 1/(1 - exp(smin-smax)),
    r2 = exp(smin-smax) * r1 -- exactly (e^s - e^smin)/(e^smax - e^smin).
  * Setup is spread across ScalarE (per-partition affine ops) and VectorE
    so the matmuls and the first exp start as early as possible.
"""

import ml_dtypes
import numpy as np

import concourse.bass as bass
import concourse.bacc as bacc
import concourse.tile as tile
from concourse import mybir
from concourse.bass_utils import run_bass_kernel_spmd

RES = 128
NPTS = RES * RES          # 16384
N_CORES = 8
G_PER_CORE = 128          # 16*64 / 8
H = 30.0 / 127.0          # grid spacing
MAGIC = 12582912.0        # 1.5 * 2**23: (x + MAGIC) - MAGIC == round(x) for |x| < 2**22
KB8 = 8                   # basis rows: q_ii, r_ii, q_ij, r_ij, q_jj, r_jj, i, j
KB = 3 * KB8              # hi/mid/lo coefficient splits

CHUNK = 2048              # ACT/DVE/DMA chunk = 4 PSUM banks
N_CHUNKS = NPTS // CHUNK  # 8
MM_N = 512                # matmul free dim = 1 PSUM bank
MM_PER_CHUNK = CHUNK // MM_N
BS_SLICE0 = 4096          # first basis slice loaded separately so MMs start early


def build_nc():
    nc = bacc.Bacc("TRN2", target_bir_lowering=False, debug=False)
    f32 = mybir.dt.float32
    bf16 = mybir.dt.bfloat16
    AOp = mybir.AluOpType
    FT = mybir.ActivationFunctionType
    X = mybir.AxisListType.X

    params_d = nc.dram_tensor("params", [G_PER_CORE, 8], f32, kind="ExternalInput")
    basis_d = nc.dram_tensor("basis", [KB, NPTS], bf16, kind="ExternalInput")
    fgrid_d = nc.dram_tensor("fgrid", [G_PER_CORE, RES + 8], f32, kind="ExternalInput")
    ident_d = nc.dram_tensor("ident", [128, 128], bf16, kind="ExternalInput")
    out_d = nc.dram_tensor("out", [G_PER_CORE, NPTS], f32, kind="ExternalOutput")
    out_ap = out_d.ap()
    basis_ap = basis_d.ap()

    with tile.TileContext(nc) as tc:
        with (
            tc.tile_pool(name="const", bufs=1) as cpool,
            tc.tile_pool(name="small", bufs=1) as sp,
            tc.tile_pool(name="mid", bufs=1) as mp,
            tc.tile_pool(name="psum", bufs=2, space=bass.MemorySpace.PSUM) as pp,
            tc.tile_pool(name="io", bufs=3) as iop,
        ):
            P = cpool.tile([128, 8], f32)
            nc.sync.dma_start(P[:], params_d.ap())
            ID = cpool.tile([128, 128], bf16)
            nc.sync.dma_start(ID[:], ident_d.ap())
            BSt = cpool.tile([KB, NPTS], bf16)
            nc.sync.dma_start(BSt[:, 0:BS_SLICE0], basis_ap[:, 0:BS_SLICE0])
            FG = cpool.tile([128, RES + 8], f32)
            nc.sync.dma_start(FG[:], fgrid_d.ap())
            nc.sync.dma_start(BSt[:, BS_SLICE0:], basis_ap[:, BS_SLICE0:])

            mux = P[:, 0:1]
            muy = P[:, 1:2]
            a = P[:, 2:3]
            b = P[:, 3:4]
            c = P[:, 4:5]
            d = P[:, 5:6]
            JG = FG[:, 0:RES]
            cornx = FG[:, RES:RES + 4]
            corny = FG[:, RES + 4:RES + 8]

            # GpSimd: zero scratch feeding the exp-table warmup (starts at t=0)
            # plus per-partition constant columns used as activation biases
            zscr = sp.tile([128, 1], f32)
            nc.gpsimd.memset(zscr[:], 0.0)
            CB = sp.tile([128, 5], f32)
            nc.gpsimd.memset(CB[:, 0:1], 15.0 / H)
            nc.gpsimd.memset(CB[:, 1:2], -15.0 / H)
            nc.gpsimd.memset(CB[:, 2:3], 127.0 + MAGIC)
            nc.gpsimd.memset(CB[:, 3:4], -MAGIC)
            nc.gpsimd.memset(CB[:, 4:5], 127.0)
            b_15h = CB[:, 0:1]
            b_n15h = CB[:, 1:2]
            b_127magic = CB[:, 2:3]
            b_nmagic = CB[:, 3:4]
            b_127 = CB[:, 4:5]

            # --- ScalarE setup chain (parallel to VectorE's) ---
            warm = sp.tile([128, 1], f32)
            nc.scalar.activation(warm[:], zscr[:], FT.Exp)   # pulls ACT_TABLE_LOAD early
            MIJ = sp.tile([128, 2], f32)    # (mi, mj) = (mu + 15)/h
            nc.scalar.activation(MIJ[:], P[:, 0:2], FT.Identity, bias=b_15h, scale=1.0 / H)
            MJI = sp.tile([128, 2], f32)    # (mj, mi) -- params cols 6,7 = (muy, mux)
            nc.scalar.activation(MJI[:], P[:, 6:8], FT.Identity, bias=b_15h, scale=1.0 / H)
            NMIJ = sp.tile([128, 2], f32)   # (-mi, -mj)
            nc.scalar.activation(NMIJ[:], P[:, 0:2], FT.Identity, bias=b_n15h, scale=-1.0 / H)
            mi = MIJ[:, 0:1]
            negmi = NMIJ[:, 0:1]
            negmj = NMIJ[:, 1:2]

            # --- VectorE: minimal chain to the matmul weights ---
            # V = (d, -b-c, a);  (Ai, Bi, Ci) = V * (-0.5 h^2 / det)
            CF = sp.tile([128, KB8], f32)
            Ai = CF[:, 1:2]
            Bi = CF[:, 3:4]
            Ci = CF[:, 5:6]
            V = sp.tile([128, 3], f32)
            nc.vector.tensor_copy(V[:, 0:1], d)
            nc.vector.scalar_tensor_tensor(V[:, 1:2], b, -1.0, c, AOp.mult, AOp.subtract)
            nc.vector.tensor_copy(V[:, 2:3], a)
            t_ad = sp.tile([128, 1], f32)
            nc.vector.tensor_mul(t_ad[:], a, d)
            ndet = sp.tile([128, 1], f32)   # b*c - a*d = -det
            nc.vector.scalar_tensor_tensor(ndet[:], b, c, t_ad[:], AOp.mult, AOp.subtract)
            nhalf = sp.tile([128, 1], f32)  # -1/det
            nc.vector.reciprocal(nhalf[:], ndet[:])
            nhi = sp.tile([128, 1], f32)    # -0.5*h^2/det
            nc.vector.tensor_scalar_mul(nhi[:], nhalf[:], 0.5 * H * H)
            nc.vector.tensor_scalar_mul(CF[:, 1:6:2], V[:, 0:3], nhi[:])   # Ai, Bi, Ci
            nc.vector.tensor_scalar_mul(CF[:, 0:6:2], CF[:, 1:6:2], 128.0)
            # dlt = -2Ai*mi - Bi*mj ; eps = -2Ci*mj - Bi*mi
            U = sp.tile([128, 2], f32)
            nc.vector.tensor_mul(U[:], CF[:, 1:6:4], MIJ[:])               # (Ai*mi, Ci*mj)
            W2 = sp.tile([128, 2], f32)
            nc.vector.tensor_scalar_mul(W2[:], MJI[:], Bi)                 # (Bi*mj, Bi*mi)
            nc.vector.scalar_tensor_tensor(CF[:, 6:8], U[:], -2.0, W2[:], AOp.mult, AOp.subtract)

            # split coefficients into hi/mid/lo bf16 (exact to fp32)
            CF24 = sp.tile([128, KB], bf16)
            nc.vector.tensor_copy(CF24[:, 0:KB8], CF[:])
            res1 = sp.tile([128, KB8], f32)
            nc.vector.tensor_sub(res1[:], CF[:], CF24[:, 0:KB8])
            nc.vector.tensor_copy(CF24[:, KB8:2 * KB8], res1[:])
            res2 = sp.tile([128, KB8], f32)
            nc.vector.tensor_sub(res2[:], res1[:], CF24[:, KB8:2 * KB8])
            nc.vector.tensor_copy(CF24[:, 2 * KB8:KB], res2[:])

            # transpose [128,24] -> lhsT [24,128] via PE; PSUM->SBUF on ScalarE
            cfT_ps = pp.tile([KB, 128], bf16, tag="ps")
            nc.tensor.transpose(cfT_ps[:], CF24[:], ID[:])
            lhsT = cpool.tile([KB, 128], bf16)
            nc.scalar.copy(lhsT[:], cfT_ps[:])

            # --- smax chain: row-argmax on ScalarE, quadratic eval on VectorE
            rA = sp.tile([128, 1], f32)
            nc.vector.reciprocal(rA[:], Ai)
            kf = sp.tile([128, 1], f32)   # -Bi/(2Ai)
            nc.vector.scalar_tensor_tensor(kf[:], Bi, -0.5, rA[:], AOp.mult, AOp.mult)

            dyj = mp.tile([128, RES], f32)
            nc.scalar.activation(dyj[:], JG, FT.Identity, bias=negmj)
            tq = mp.tile([128, RES], f32)
            nc.scalar.activation(tq[:], dyj[:], FT.Identity, bias=mi, scale=kf[:])
            tq1 = mp.tile([128, RES], f32)   # max(tq, 0)
            nc.scalar.activation(tq1[:], tq[:], FT.Relu)
            tq2 = mp.tile([128, RES], f32)   # max(127 - tq1, 0)
            nc.scalar.activation(tq2[:], tq1[:], FT.Relu, bias=b_127, scale=-1.0)
            tq3 = mp.tile([128, RES], f32)   # clamp + MAGIC, rounds to int
            nc.scalar.activation(tq3[:], tq2[:], FT.Identity, bias=b_127magic, scale=-1.0)
            tqr = mp.tile([128, RES], f32)   # round(clamp(tq))
            nc.scalar.activation(tqr[:], tq3[:], FT.Identity, bias=b_nmagic)
            dqi = mp.tile([128, RES], f32)   # i_q - mi
            nc.scalar.activation(dqi[:], tqr[:], FT.Identity, bias=negmi)

            w1 = mp.tile([128, RES], f32)
            nc.vector.tensor_scalar_mul(w1[:], dyj[:], Ci)
            w2 = mp.tile([128, RES], f32)
            nc.vector.scalar_tensor_tensor(w2[:], dqi[:], Bi, w1[:], AOp.mult, AOp.add)
            w3 = mp.tile([128, RES], f32)
            nc.vector.tensor_mul(w3[:], w2[:], dyj[:])
            w4 = mp.tile([128, RES], f32)
            nc.vector.scalar_tensor_tensor(w4[:], dqi[:], Ai, dqi[:], AOp.mult, AOp.mult)
            mrow = mp.tile([128, RES], f32)
            nc.vector.tensor_add(mrow[:], w3[:], w4[:])
            smax = sp.tile([128, 1], f32)
            nc.vector.tensor_reduce(smax[:], mrow[:], X, AOp.max)

            # c0 = Ai*mi^2 + Bi*mi*mj + Ci*mj^2
            m12 = sp.tile([128, 2], f32)    # (Ai*mi^2, Ci*mj^2)
            nc.vector.tensor_mul(m12[:], U[:], MIJ[:])
            m3 = sp.tile([128, 1], f32)     # Bi*mj * mi
            nc.vector.tensor_mul(m3[:], W2[:, 0:1], mi)
            c0 = sp.tile([128, 1], f32)
            nc.vector.tensor_add(c0[:], m12[:, 0:1], m12[:, 1:2])
            nc.vector.tensor_add(c0[:], c0[:], m3[:])
            ebias = sp.tile([128, 1], f32)   # c0 - smax  (exp input bias)
            nc.vector.tensor_sub(ebias[:], c0[:], smax[:])

            # smin: corners on ScalarE affine + VectorE quadratic
            dxc = sp.tile([128, 4], f32)
            nc.scalar.activation(dxc[:], cornx, FT.Identity, bias=negmi)
            dyc = sp.tile([128, 4], f32)
            nc.scalar.activation(dyc[:], corny, FT.Identity, bias=negmj)
            z1 = sp.tile([128, 4], f32)
            nc.vector.tensor_scalar_mul(z1[:], dyc[:], Ci)
            z2 = sp.tile([128, 4], f32)
            nc.vector.scalar_tensor_tensor(z2[:], dxc[:], Bi, z1[:], AOp.mult, AOp.add)
            z3 = sp.tile([128, 4], f32)
            nc.vector.tensor_mul(z3[:], z2[:], dyc[:])
            z4 = sp.tile([128, 4], f32)
            nc.vector.scalar_tensor_tensor(z4[:], dxc[:], Ai, dxc[:], AOp.mult, AOp.mult)
            zm = sp.tile([128, 4], f32)
            nc.vector.tensor_add(zm[:], z3[:], z4[:])
            smin = sp.tile([128, 1], f32)
            nc.vector.tensor_reduce(smin[:], zm[:], X, AOp.min)

            # normalization scalars
            tdiff = sp.tile([128, 1], f32)
            nc.vector.tensor_sub(tdiff[:], smin[:], smax[:])
            et = sp.tile([128, 1], f32)
            nc.scalar.activation(et[:], tdiff[:], FT.Exp)
            om = sp.tile([128, 1], f32)   # 1 - et
            nc.vector.tensor_scalar(om[:], et[:], -1.0, 1.0, AOp.mult, AOp.add)
            r1 = sp.tile([128, 1], f32)
            nc.vector.reciprocal(r1[:], om[:])
            r2 = sp.tile([128, 1], f32)
            nc.vector.tensor_mul(r2[:], et[:], r1[:])

            # --- main loop: matmul -> exp(+bias) -> normalize -> DMA out ---
            for ch in range(N_CHUNKS):
                ps = pp.tile([128, CHUNK], f32, tag="ps")
                for mm in range(MM_PER_CHUNK):
                    lo = ch * CHUNK + mm * MM_N
                    nc.tensor.matmul(
                        ps[:, mm * MM_N:(mm + 1) * MM_N],
                        lhsT[:],
                        BSt[:, lo:lo + MM_N],
                        start=True,
                        stop=True,
                    )
                e = iop.tile([128, CHUNK], f32, tag="e")
                nc.scalar.activation(e[:], ps[:], FT.Exp, bias=ebias[:])
                o = iop.tile([128, CHUNK], f32, tag="o")
                nc.vector.tensor_scalar(o[:], e[:], r1[:], r2[:], AOp.mult, AOp.subtract)
                nc.sync.dma_start(out_ap[:, ch * CHUNK:(ch + 1) * CHUNK], o[:])

    nc.compile()
    return nc


def make_constants():
    idx = np.arange(RES, dtype=np.int64)
    i = np.tile(idx, RES)                        # flat idx n = j*RES + i
    j = np.repeat(idx, RES)
    rows8 = []
    for prod in (i * i, i * j, j * j):
        rows8.append(prod // 128)                # q < 128
        rows8.append(prod % 128)                 # r < 128
    rows8.append(i)
    rows8.append(j)
    basis8 = np.stack(rows8).astype(np.float64)  # all small ints, exact in bf16
    basis = np.concatenate([basis8, basis8, basis8]).astype(ml_dtypes.bfloat16)
    fgrid = np.zeros((G_PER_CORE, RES + 8), np.float32)
    fgrid[:, 0:RES] = idx.astype(np.float32)[None, :]
    fgrid[:, RES:RES + 4] = np.array([0.0, 127.0, 0.0, 127.0], np.float32)
    fgrid[:, RES + 4:RES + 8] = np.array([0.0, 0.0, 127.0, 127.0], np.float32)
    ident = np.eye(128, dtype=ml_dtypes.bfloat16)
    return basis, fgrid, ident


def make_in_maps(mu, covar):
    mu = np.ascontiguousarray(np.asarray(mu), dtype=np.float32)
    covar = np.ascontiguousarray(np.asarray(covar), dtype=np.float32)
    G = mu.shape[0] * mu.shape[1]
    muf = mu.reshape(G, 2)
    cvf = covar.reshape(G, 4)
    basis, fgrid, ident = make_constants()
    in_maps = []
    for cid in range(N_CORES):
        sl = slice(cid * G_PER_CORE, (cid + 1) * G_PER_CORE)
        params = np.zeros((G_PER_CORE, 8), np.float32)
        params[:, 0] = muf[sl, 0]
        params[:, 1] = muf[sl, 1]
        params[:, 2] = cvf[sl, 0]   # a
        params[:, 3] = cvf[sl, 1]   # b
        params[:, 4] = cvf[sl, 2]   # c
        params[:, 5] = cvf[sl, 3]   # d
        params[:, 6] = muf[sl, 1]   # muy again (for the (mj, mi) affine)
        params[:, 7] = muf[sl, 0]   # mux again
        in_maps.append(
            {
                "params": params,
                "basis": basis,
                "fgrid": fgrid,
                "ident": ident,
            }
        )
    return in_maps


_NC_CACHE = None


def get_nc():
    global _NC_CACHE
    if _NC_CACHE is None:
        _NC_CACHE = build_nc()
    return _NC_CACHE


def kernel(mu, covar, _trace=False, _trace_kwargs=None):
    in_maps = make_in_maps(mu, covar)
    nc = get_nc()
    res = run_bass_kernel_spmd(
        nc, in_maps, core_ids=list(range(N_CORES)), trace=_trace,
        **(_trace_kwargs or {}),
    )
    outs = [np.asarray(res.results[i]["out"]) for i in range(N_CORES)]
    full = np.concatenate(outs, axis=0)           # [1024, 16384]
    out = full.reshape(16, 64, 1, RES, RES).astype(np.float32, copy=False)
    if _trace:
        return out, res
    return out


# revision 12
# speedup vs baseline: 1.7698x; 1.0294x over previous
"""Trainium2 Bass kernel for nn_AnalyticalDecoder.

Evaluates 1024 2-D Gaussians (BS=16 x T=64) on a fixed 128x128 grid and
min/max-normalizes each Gaussian's field.  Output [16,64,1,128,128] f32.

Strategy (data-parallel over the 8 NeuronCores, 128 Gaussians per core,
one Gaussian per SBUF partition):
  * Work in grid-index coordinates (i, j in 0..127).  s(g, j, i) =
    -0.5 (p-mu)^T Sigma^-1 (p-mu) is a quadratic in (i, j), so the
    16384-point field per Gaussian is one matmul against the constant basis
    {i^2, i*j, j^2, i, j, 1} (TensorE), exp on ScalarE, affine normalize on
    VectorE: out = exp(s) * R1 - r2 with R1 = 1/(e^smax - e^smin),
    r2 = e^smin * R1 -- exactly (e^s - e^smin)/(e^smax - e^smin).  The
    min/max normalization therefore only gates the (cheap, late) VectorE
    stage, never the matmul/exp pipeline.
  * Precision trick: fp32 matmul is 4-5 cyc/row and fp32r loses ~12 mantissa
    bits (fatal in the monomial cancellation near the peak).  Instead each
    integer basis product is split exactly as v = 128*q + r with q, r < 128
    -- exactly representable in bf16 -- and each of the 9 coefficients is
    split into hi/mid/lo bf16 parts (24 mantissa bits).  The K=27 bf16
    matmul streams at 1 cyc/row (K is free on the 128-deep PE), with exact
    basis values and fp32-accurate coefficients.
  * The normalization prefactor exp(-log2pi - 0.5 log det) cancels in
    (p - mn)/(mx - mn), so only s is needed.
  * min over the grid of the concave quadratic s is attained exactly at one
    of the 4 grid corners.  max: for each row j the restriction to i is a
    concave parabola whose discrete argmax is the grid point nearest its
    vertex (closed form), then a 128-point reduce over rows.  This avoids
    two full 16K-element reduction passes per partition.
  * Setup is spread across ScalarE (per-partition affine ops), GpSimd
    (corner quadratic) and VectorE so every pipeline stage starts ASAP.
"""

import ml_dtypes
import numpy as np

import concourse.bass as bass
import concourse.bacc as bacc
import concourse.tile as tile
from concourse import mybir
from concourse.bass_utils import run_bass_kernel_spmd

RES = 128
NPTS = RES * RES          # 16384
N_CORES = 8
G_PER_CORE = 128          # 16*64 / 8
H = 30.0 / 127.0          # grid spacing
MAGIC = 12582912.0        # 1.5 * 2**23: (x + MAGIC) - MAGIC == round(x) for |x| < 2**22
KB9 = 9                   # basis rows: q_ii, r_ii, q_ij, r_ij, q_jj, r_jj, i, j, 1
KB = 3 * KB9              # hi/mid/lo coefficient splits

CHUNK = 1024              # ACT/DVE/DMA chunk = 2 PSUM banks
N_CHUNKS = NPTS // CHUNK  # 8
MM_N = 512                # matmul free dim = 1 PSUM bank
MM_PER_CHUNK = CHUNK // MM_N
BS_SLICE0 = 4096          # first basis slice loaded separately so MMs start early


def build_nc():
    nc = bacc.Bacc("TRN2", target_bir_lowering=False, debug=False)
    f32 = mybir.dt.float32
    bf16 = mybir.dt.bfloat16
    AOp = mybir.AluOpType
    FT = mybir.ActivationFunctionType
    X = mybir.AxisListType.X

    params_d = nc.dram_tensor("params", [G_PER_CORE, 8], f32, kind="ExternalInput")
    basis_d = nc.dram_tensor("basis", [KB, NPTS], bf16, kind="ExternalInput")
    fgrid_d = nc.dram_tensor("fgrid", [G_PER_CORE, RES + 8], f32, kind="ExternalInput")
    ident_d = nc.dram_tensor("ident", [128, 128], bf16, kind="ExternalInput")
    out_d = nc.dram_tensor("out", [G_PER_CORE, NPTS], f32, kind="ExternalOutput")
    out_ap = out_d.ap()
    basis_ap = basis_d.ap()

    with tile.TileContext(nc) as tc:
        with (
            tc.tile_pool(name="const", bufs=1) as cpool,
            tc.tile_pool(name="small", bufs=1) as sp,
            tc.tile_pool(name="mid", bufs=1) as mp,
            tc.tile_pool(name="psum", bufs=2, space=bass.MemorySpace.PSUM) as pp,
            tc.tile_pool(name="io", bufs=3) as iop,
        ):
            P = cpool.tile([128, 8], f32)
            nc.sync.dma_start(P[:], params_d.ap())
            BSt = cpool.tile([KB, NPTS], bf16)
            nc.sync.dma_start(BSt[:, 0:BS_SLICE0], basis_ap[:, 0:BS_SLICE0])
            ID = cpool.tile([128, 128], bf16)
            nc.sync.dma_start(ID[:], ident_d.ap())
            FG = cpool.tile([128, RES + 8], f32)
            nc.sync.dma_start(FG[:], fgrid_d.ap())
            nc.sync.dma_start(BSt[:, BS_SLICE0:], basis_ap[:, BS_SLICE0:])

            a = P[:, 2:3]
            b = P[:, 3:4]
            c = P[:, 4:5]
            d = P[:, 5:6]
            JG = FG[:, 0:RES]
            cornx = FG[:, RES:RES + 4]
            corny = FG[:, RES + 4:RES + 8]

            # GpSimd: zero scratch feeding the exp-table warmup (starts at t=0)
            # plus per-partition constant columns used as activation biases
            zscr = sp.tile([128, 1], f32)
            nc.gpsimd.memset(zscr[:], 0.0)
            CB = sp.tile([128, 2], f32)
            nc.gpsimd.memset(CB[:, 0:1], 15.0 / H)
            nc.gpsimd.memset(CB[:, 1:2], -15.0 / H)
            b_15h = CB[:, 0:1]
            b_n15h = CB[:, 1:2]

            # --- ScalarE setup chain (parallel to VectorE's) ---
            warm = sp.tile([128, 1], f32)
            nc.scalar.activation(warm[:], zscr[:], FT.Exp)   # pulls ACT_TABLE_LOAD early
            MIJ = sp.tile([128, 2], f32)    # (mi, mj) = (mu + 15)/h
            nc.scalar.activation(MIJ[:], P[:, 0:2], FT.Identity, bias=b_15h, scale=1.0 / H)
            MJI = sp.tile([128, 2], f32)    # (mj, mi) -- params cols 6,7 = (muy, mux)
            nc.scalar.activation(MJI[:], P[:, 6:8], FT.Identity, bias=b_15h, scale=1.0 / H)
            NMIJ = sp.tile([128, 2], f32)   # (-mi, -mj)
            nc.scalar.activation(NMIJ[:], P[:, 0:2], FT.Identity, bias=b_n15h, scale=-1.0 / H)
            mi = MIJ[:, 0:1]
            negmi = NMIJ[:, 0:1]
            negmj = NMIJ[:, 1:2]
            dyj = mp.tile([128, RES], f32)  # j - mj
            nc.scalar.activation(dyj[:], JG, FT.Identity, bias=negmj)
            dxc = sp.tile([128, 4], f32)    # corner i - mi
            nc.scalar.activation(dxc[:], cornx, FT.Identity, bias=negmi)
            dyc = sp.tile([128, 4], f32)    # corner j - mj
            nc.scalar.activation(dyc[:], corny, FT.Identity, bias=negmj)

            # --- VectorE: minimal chain to the matmul weights ---
            # V = (d, -b-c, a);  (Ai, Bi, Ci) = V * (-0.5 h^2 / det)
            CF = sp.tile([128, KB9], f32)
            Ai = CF[:, 1:2]
            Bi = CF[:, 3:4]
            Ci = CF[:, 5:6]
            V = sp.tile([128, 3], f32)
            nc.vector.tensor_copy(V[:, 0:1], d)
            nc.vector.scalar_tensor_tensor(V[:, 1:2], b, -1.0, c, AOp.mult, AOp.subtract)
            nc.vector.tensor_copy(V[:, 2:3], a)
            t_ad = sp.tile([128, 1], f32)
            nc.vector.tensor_mul(t_ad[:], a, d)
            ndet = sp.tile([128, 1], f32)   # b*c - a*d = -det
            nc.vector.scalar_tensor_tensor(ndet[:], b, c, t_ad[:], AOp.mult, AOp.subtract)
            nhalf = sp.tile([128, 1], f32)  # -1/det
            nc.vector.reciprocal(nhalf[:], ndet[:])
            nhi = sp.tile([128, 1], f32)    # -0.5*h^2/det
            nc.vector.tensor_scalar_mul(nhi[:], nhalf[:], 0.5 * H * H)
            nc.vector.tensor_scalar_mul(CF[:, 1:6:2], V[:, 0:3], nhi[:])   # Ai, Bi, Ci
            nc.vector.tensor_scalar_mul(CF[:, 0:6:2], CF[:, 1:6:2], 128.0)
            # dlt = -2Ai*mi - Bi*mj ; eps = -2Ci*mj - Bi*mi
            U = sp.tile([128, 2], f32)
            nc.vector.tensor_mul(U[:], CF[:, 1:6:4], MIJ[:])               # (Ai*mi, Ci*mj)
            W2 = sp.tile([128, 2], f32)
            nc.vector.tensor_scalar_mul(W2[:], MJI[:], Bi)                 # (Bi*mj, Bi*mi)
            nc.vector.scalar_tensor_tensor(CF[:, 6:8], U[:], -2.0, W2[:], AOp.mult, AOp.subtract)
            # c0 = Ai*mi^2 + Bi*mi*mj + Ci*mj^2  (constant basis row)
            m12 = sp.tile([128, 2], f32)    # (Ai*mi^2, Ci*mj^2)
            nc.vector.tensor_mul(m12[:], U[:], MIJ[:])
            m3 = sp.tile([128, 1], f32)     # (Bi*mj) * mi
            nc.vector.tensor_mul(m3[:], W2[:, 0:1], mi)
            nc.vector.tensor_add(m3[:], m3[:], m12[:, 0:1])
            nc.vector.tensor_add(CF[:, 8:9], m3[:], m12[:, 1:2])
            # row-argmax slope while we're here (gates the smax chain below)
            rA = sp.tile([128, 1], f32)
            nc.vector.reciprocal(rA[:], Ai)
            kf = sp.tile([128, 1], f32)     # -Bi/(2Ai)
            nc.vector.scalar_tensor_tensor(kf[:], Bi, -0.5, rA[:], AOp.mult, AOp.mult)

            # split coefficients into hi/mid/lo bf16 (exact to fp32)
            CF27 = sp.tile([128, KB], bf16)
            nc.vector.tensor_copy(CF27[:, 0:KB9], CF[:])
            res1 = sp.tile([128, KB9], f32)
            nc.vector.tensor_sub(res1[:], CF[:], CF27[:, 0:KB9])
            nc.vector.tensor_copy(CF27[:, KB9:2 * KB9], res1[:])
            res2 = sp.tile([128, KB9], f32)
            nc.vector.tensor_sub(res2[:], res1[:], CF27[:, KB9:2 * KB9])
            nc.vector.tensor_copy(CF27[:, 2 * KB9:KB], res2[:])

            # transpose [128,27] -> lhsT [27,128] via PE; PSUM->SBUF on ScalarE
            cfT_ps = pp.tile([KB, 128], bf16, tag="ps")
            nc.tensor.transpose(cfT_ps[:], CF27[:], ID[:])
            lhsT = cpool.tile([KB, 128], bf16)
            nc.scalar.copy(lhsT[:], cfT_ps[:])

            # --- smax: row-argmax in closed form, then 128-point reduce ---
            tq = mp.tile([128, RES], f32)   # continuous col index of row-argmax
            nc.vector.tensor_scalar(tq[:], dyj[:], kf[:], mi, AOp.mult, AOp.add)
            tqc = mp.tile([128, RES], f32)
            nc.vector.tensor_scalar(tqc[:], tq[:], 0.0, 127.0, AOp.max, AOp.min)
            tqr = mp.tile([128, RES], f32)  # round to nearest grid index
            nc.vector.tensor_scalar(tqr[:], tqc[:], MAGIC, MAGIC, AOp.add, AOp.subtract)
            dqi = mp.tile([128, RES], f32)  # i_q - mi
            nc.vector.tensor_scalar_sub(dqi[:], tqr[:], mi)
            w1 = mp.tile([128, RES], f32)
            nc.vector.tensor_scalar_mul(w1[:], dyj[:], Ci)
            w2 = mp.tile([128, RES], f32)
            nc.vector.scalar_tensor_tensor(w2[:], dqi[:], Bi, w1[:], AOp.mult, AOp.add)
            w3 = mp.tile([128, RES], f32)
            nc.vector.tensor_mul(w3[:], w2[:], dyj[:])
            w4 = mp.tile([128, RES], f32)
            nc.vector.scalar_tensor_tensor(w4[:], dqi[:], Ai, dqi[:], AOp.mult, AOp.mult)
            mrow = mp.tile([128, RES], f32)
            nc.vector.tensor_add(mrow[:], w3[:], w4[:])
            SMM = sp.tile([128, 2], f32)    # (smax, smin)
            nc.vector.tensor_reduce(SMM[:, 0:1], mrow[:], X, AOp.max)

            # --- smin: corner quadratic (s concave -> min at a grid corner)
            z1 = sp.tile([128, 4], f32)
            nc.vector.tensor_scalar_mul(z1[:], dyc[:], Ci)
            z2 = sp.tile([128, 4], f32)
            nc.vector.scalar_tensor_tensor(z2[:], dxc[:], Bi, z1[:], AOp.mult, AOp.add)
            z3 = sp.tile([128, 4], f32)
            nc.vector.tensor_mul(z3[:], z2[:], dyc[:])
            z4 = sp.tile([128, 4], f32)
            nc.vector.scalar_tensor_tensor(z4[:], dxc[:], Ai, dxc[:], AOp.mult, AOp.mult)
            zm = sp.tile([128, 4], f32)
            nc.vector.tensor_add(zm[:], z3[:], z4[:])
            nc.vector.tensor_reduce(SMM[:, 1:2], zm[:], X, AOp.min)

            # normalization scalars: R1 = 1/(e^smax - e^smin), r2 = e^smin * R1
            ESM = sp.tile([128, 2], f32)
            nc.scalar.activation(ESM[:], SMM[:], FT.Exp)
            Dn = sp.tile([128, 1], f32)
            nc.vector.tensor_sub(Dn[:], ESM[:, 0:1], ESM[:, 1:2])
            R1 = sp.tile([128, 1], f32)
            nc.vector.reciprocal(R1[:], Dn[:])
            r2 = sp.tile([128, 1], f32)
            nc.vector.tensor_mul(r2[:], ESM[:, 1:2], R1[:])

            # --- main loop: matmul -> exp -> normalize -> DMA out ---
            for ch in range(N_CHUNKS):
                ps = pp.tile([128, CHUNK], f32, tag="ps")
                for mm in range(MM_PER_CHUNK):
                    lo = ch * CHUNK + mm * MM_N
                    nc.tensor.matmul(
                        ps[:, mm * MM_N:(mm + 1) * MM_N],
                        lhsT[:],
                        BSt[:, lo:lo + MM_N],
                        start=True,
                        stop=True,
                    )
                e = iop.tile([128, CHUNK], f32, tag="e")
                nc.scalar.activation(e[:], ps[:], FT.Exp)
                o = iop.tile([128, CHUNK], f32, tag="o")
                nc.vector.tensor_scalar(o[:], e[:], R1[:], r2[:], AOp.mult, AOp.subtract)
                nc.sync.dma_start(out_ap[:, ch * CHUNK:(ch + 1) * CHUNK], o[:])

    nc.compile()
    return nc


def make_constants():
    idx = np.arange(RES, dtype=np.int64)
    i = np.tile(idx, RES)                        # flat idx n = j*RES + i
    j = np.repeat(idx, RES)
    rows9 = []
    for prod in (i * i, i * j, j * j):
        rows9.append(prod // 128)                # q < 128
        rows9.append(prod % 128)                 # r < 128
    rows9.append(i)
    rows9.append(j)
    rows9.append(np.ones(NPTS, dtype=np.int64))
    basis9 = np.stack(rows9).astype(np.float64)  # all small ints, exact in bf16
    basis = np.concatenate([basis9, basis9, basis9]).astype(ml_dtypes.bfloat16)
    fgrid = np.zeros((G_PER_CORE, RES + 8), np.float32)
    fgrid[:, 0:RES] = idx.astype(np.float32)[None, :]
    fgrid[:, RES:RES + 4] = np.array([0.0, 127.0, 0.0, 127.0], np.float32)
    fgrid[:, RES + 4:RES + 8] = np.array([0.0, 0.0, 127.0, 127.0], np.float32)
    ident = np.eye(128, dtype=ml_dtypes.bfloat16)
    return basis, fgrid, ident


def make_in_maps(mu, covar):
    mu = np.ascontiguousarray(np.asarray(mu), dtype=np.float32)
    covar = np.ascontiguousarray(np.asarray(covar), dtype=np.float32)
    G = mu.shape[0] * mu.shape[1]
    muf = mu.reshape(G, 2)
    cvf = covar.reshape(G, 4)
    basis, fgrid, ident = make_constants()
    in_maps = []
    for cid in range(N_CORES):
        sl = slice(cid * G_PER_CORE, (cid + 1) * G_PER_CORE)
        params = np.zeros((G_PER_CORE, 8), np.float32)
        params[:, 0] = muf[sl, 0]
        params[:, 1] = muf[sl, 1]
        params[:, 2] = cvf[sl, 0]   # a
        params[:, 3] = cvf[sl, 1]   # b
        params[:, 4] = cvf[sl, 2]   # c
        params[:, 5] = cvf[sl, 3]   # d
        params[:, 6] = muf[sl, 1]   # muy again (for the (mj, mi) affine)
        params[:, 7] = muf[sl, 0]   # mux again
        in_maps.append(
            {
                "params": params,
                "basis": basis,
                "fgrid": fgrid,
                "ident": ident,
            }
        )
    return in_maps


_NC_CACHE = None


def get_nc():
    global _NC_CACHE
    if _NC_CACHE is None:
        _NC_CACHE = build_nc()
    return _NC_CACHE


def kernel(mu, covar, _trace=False, _trace_kwargs=None):
    in_maps = make_in_maps(mu, covar)
    nc = get_nc()
    res = run_bass_kernel_spmd(
        nc, in_maps, core_ids=list(range(N_CORES)), trace=_trace,
        **(_trace_kwargs or {}),
    )
    outs = [np.asarray(res.results[i]["out"]) for i in range(N_CORES)]
    full = np.concatenate(outs, axis=0)           # [1024, 16384]
    out = full.reshape(16, 64, 1, RES, RES).astype(np.float32, copy=False)
    if _trace:
        return out, res
    return out


# revision 13
# speedup vs baseline: 1.9968x; 1.1283x over previous
"""Trainium2 Bass kernel for nn_AnalyticalDecoder.

Evaluates 1024 2-D Gaussians (BS=16 x T=64) on a fixed 128x128 grid and
min/max-normalizes each Gaussian's field.  Output [16,64,1,128,128] f32.

Strategy (data-parallel over the 8 NeuronCores, 128 Gaussians per core,
one Gaussian per SBUF partition):
  * Work in grid-index coordinates (i, j in 0..127).  s(g, j, i) =
    -0.5 (p-mu)^T Sigma^-1 (p-mu) is a quadratic in (i, j), so the
    16384-point field per Gaussian is one matmul against the constant basis
    {i^2, i*j, j^2, i, j, 1} (TensorE), exp on ScalarE, affine normalize on
    VectorE: out = exp(s) * R1 - r2 with R1 = 1/(e^smax - e^smin),
    r2 = e^smin * R1 -- exactly (e^s - e^smin)/(e^smax - e^smin).  The
    min/max normalization therefore only gates the (cheap, late) VectorE
    stage, never the matmul/exp pipeline.
  * Precision trick: fp32 matmul is 4-5 cyc/row and fp32r loses ~12 mantissa
    bits (fatal in the monomial cancellation near the peak).  Instead each
    integer basis product is split exactly as v = 128*q + r with q, r < 128
    -- exactly representable in bf16 -- and each of the 9 coefficients is
    split into hi/mid/lo bf16 parts (24 mantissa bits).  The K=27 bf16
    matmul streams at 1 cyc/row (K is free on the 128-deep PE), with exact
    basis values and fp32-accurate coefficients.
  * The normalization prefactor exp(-log2pi - 0.5 log det) cancels in
    (p - mn)/(mx - mn), so only s is needed.
  * min over the grid of the concave quadratic s is attained exactly at one
    of the 4 grid corners.  max: for each row j the restriction to i is a
    concave parabola whose discrete argmax is the grid point nearest its
    vertex (closed form), then a 128-point reduce over rows.  This avoids
    two full 16K-element reduction passes per partition.
  * Setup is spread across ScalarE (per-partition affine ops), GpSimd
    (corner quadratic) and VectorE so every pipeline stage starts ASAP.
"""

import ml_dtypes
import numpy as np

import concourse.bass as bass
import concourse.bacc as bacc
import concourse.tile as tile
from concourse import mybir
from concourse.bass_utils import run_bass_kernel_spmd

RES = 128
NPTS = RES * RES          # 16384
N_CORES = 8
G_PER_CORE = 128          # 16*64 / 8
H = 30.0 / 127.0          # grid spacing
MAGIC = 12582912.0        # 1.5 * 2**23: (x + MAGIC) - MAGIC == round(x) for |x| < 2**22
KB9 = 9                   # basis rows: q_ii, r_ii, q_ij, r_ij, q_jj, r_jj, i, j, 1
KB = 3 * KB9              # hi/mid/lo coefficient splits

CHUNK = 2048              # ACT/DVE/DMA chunk = 4 PSUM banks
N_CHUNKS = NPTS // CHUNK  # 8
MM_N = 512                # matmul free dim = 1 PSUM bank
MM_PER_CHUNK = CHUNK // MM_N
BS_SLICE0 = 4096          # first basis slice loaded separately so MMs start early


def build_nc():
    nc = bacc.Bacc("TRN2", target_bir_lowering=False, debug=False)
    f32 = mybir.dt.float32
    bf16 = mybir.dt.bfloat16
    AOp = mybir.AluOpType
    FT = mybir.ActivationFunctionType
    X = mybir.AxisListType.X

    params_d = nc.dram_tensor("params", [G_PER_CORE, 8], f32, kind="ExternalInput")
    basis_d = nc.dram_tensor("basis", [KB, NPTS], bf16, kind="ExternalInput")
    fgrid_d = nc.dram_tensor("fgrid", [G_PER_CORE, RES + 8], f32, kind="ExternalInput")
    ident_d = nc.dram_tensor("ident", [128, 128], bf16, kind="ExternalInput")
    out_d = nc.dram_tensor("out", [G_PER_CORE, NPTS], f32, kind="ExternalOutput")
    out_ap = out_d.ap()
    basis_ap = basis_d.ap()

    with tile.TileContext(nc) as tc:
        with (
            tc.tile_pool(name="const", bufs=1) as cpool,
            tc.tile_pool(name="small", bufs=1) as sp,
            tc.tile_pool(name="mid", bufs=1) as mp,
            tc.tile_pool(name="psum", bufs=2, space=bass.MemorySpace.PSUM) as pp,
            tc.tile_pool(name="io", bufs=3) as iop,
        ):
            P = cpool.tile([128, 8], f32)
            nc.sync.dma_start(P[:], params_d.ap())
            BSt = cpool.tile([KB, NPTS], bf16)
            nc.sync.dma_start(BSt[:, 0:BS_SLICE0], basis_ap[:, 0:BS_SLICE0])
            ID = cpool.tile([128, 128], bf16)
            nc.sync.dma_start(ID[:], ident_d.ap())
            FG = cpool.tile([128, RES + 8], f32)
            nc.sync.dma_start(FG[:], fgrid_d.ap())
            nc.sync.dma_start(BSt[:, BS_SLICE0:], basis_ap[:, BS_SLICE0:])

            a = P[:, 2:3]
            b = P[:, 3:4]
            c = P[:, 4:5]
            d = P[:, 5:6]
            JG = FG[:, 0:RES]
            cornx = FG[:, RES:RES + 4]
            corny = FG[:, RES + 4:RES + 8]

            # GpSimd: zero scratch feeding the exp-table warmup (starts at t=0)
            # plus per-partition constant columns used as activation biases
            zscr = sp.tile([128, 1], f32)
            nc.gpsimd.memset(zscr[:], 0.0)
            CB = sp.tile([128, 2], f32)
            nc.gpsimd.memset(CB[:, 0:1], 15.0 / H)
            nc.gpsimd.memset(CB[:, 1:2], -15.0 / H)
            b_15h = CB[:, 0:1]
            b_n15h = CB[:, 1:2]

            # --- ScalarE setup chain (parallel to VectorE's) ---
            warm = sp.tile([128, 1], f32)
            nc.scalar.activation(warm[:], zscr[:], FT.Exp)   # pulls ACT_TABLE_LOAD early
            MIJ = sp.tile([128, 2], f32)    # (mi, mj) = (mu + 15)/h
            nc.scalar.activation(MIJ[:], P[:, 0:2], FT.Identity, bias=b_15h, scale=1.0 / H)
            MJI = sp.tile([128, 2], f32)    # (mj, mi) -- params cols 6,7 = (muy, mux)
            nc.scalar.activation(MJI[:], P[:, 6:8], FT.Identity, bias=b_15h, scale=1.0 / H)
            NMIJ = sp.tile([128, 2], f32)   # (-mi, -mj)
            nc.scalar.activation(NMIJ[:], P[:, 0:2], FT.Identity, bias=b_n15h, scale=-1.0 / H)
            mi = MIJ[:, 0:1]
            negmi = NMIJ[:, 0:1]
            negmj = NMIJ[:, 1:2]
            dyj = mp.tile([128, RES], f32)  # j - mj
            nc.scalar.activation(dyj[:], JG, FT.Identity, bias=negmj)
            dxc = sp.tile([128, 4], f32)    # corner i - mi
            nc.scalar.activation(dxc[:], cornx, FT.Identity, bias=negmi)
            dyc = sp.tile([128, 4], f32)    # corner j - mj
            nc.scalar.activation(dyc[:], corny, FT.Identity, bias=negmj)

            # --- VectorE: minimal chain to the matmul weights ---
            # V = (d, -b-c, a);  (Ai, Bi, Ci) = V * (-0.5 h^2 / det)
            CF = sp.tile([128, KB9], f32)
            Ai = CF[:, 1:2]
            Bi = CF[:, 3:4]
            Ci = CF[:, 5:6]
            V = sp.tile([128, 3], f32)
            nc.vector.tensor_copy(V[:, 0:1], d)
            nc.vector.scalar_tensor_tensor(V[:, 1:2], b, -1.0, c, AOp.mult, AOp.subtract)
            nc.vector.tensor_copy(V[:, 2:3], a)
            t_ad = sp.tile([128, 1], f32)
            nc.vector.tensor_mul(t_ad[:], a, d)
            ndet = sp.tile([128, 1], f32)   # b*c - a*d = -det
            nc.vector.scalar_tensor_tensor(ndet[:], b, c, t_ad[:], AOp.mult, AOp.subtract)
            nhalf = sp.tile([128, 1], f32)  # -1/det
            nc.vector.reciprocal(nhalf[:], ndet[:])
            nhi = sp.tile([128, 1], f32)    # -0.5*h^2/det
            nc.vector.tensor_scalar_mul(nhi[:], nhalf[:], 0.5 * H * H)
            nc.vector.tensor_scalar_mul(CF[:, 1:6:2], V[:, 0:3], nhi[:])   # Ai, Bi, Ci
            nc.vector.tensor_scalar_mul(CF[:, 0:6:2], CF[:, 1:6:2], 128.0)
            # dlt = -2Ai*mi - Bi*mj ; eps = -2Ci*mj - Bi*mi
            U = sp.tile([128, 2], f32)
            nc.vector.tensor_mul(U[:], CF[:, 1:6:4], MIJ[:])               # (Ai*mi, Ci*mj)
            W2 = sp.tile([128, 2], f32)
            nc.vector.tensor_scalar_mul(W2[:], MJI[:], Bi)                 # (Bi*mj, Bi*mi)
            nc.vector.scalar_tensor_tensor(CF[:, 6:8], U[:], -2.0, W2[:], AOp.mult, AOp.subtract)
            # c0 = Ai*mi^2 + Bi*mi*mj + Ci*mj^2  (constant basis row)
            m12 = sp.tile([128, 2], f32)    # (Ai*mi^2, Ci*mj^2)
            nc.vector.tensor_mul(m12[:], U[:], MIJ[:])
            m3 = sp.tile([128, 1], f32)     # (Bi*mj) * mi
            nc.vector.tensor_mul(m3[:], W2[:, 0:1], mi)
            nc.vector.tensor_add(m3[:], m3[:], m12[:, 0:1])
            nc.vector.tensor_add(CF[:, 8:9], m3[:], m12[:, 1:2])
            # row-argmax slope while we're here (gates the smax chain below)
            rA = sp.tile([128, 1], f32)
            nc.vector.reciprocal(rA[:], Ai)
            kf = sp.tile([128, 1], f32)     # -Bi/(2Ai)
            nc.vector.scalar_tensor_tensor(kf[:], Bi, -0.5, rA[:], AOp.mult, AOp.mult)

            # split coefficients into hi/mid/lo bf16 (exact to fp32)
            CF27 = sp.tile([128, KB], bf16)
            nc.vector.tensor_copy(CF27[:, 0:KB9], CF[:])
            res1 = sp.tile([128, KB9], f32)
            nc.vector.tensor_sub(res1[:], CF[:], CF27[:, 0:KB9])
            nc.vector.tensor_copy(CF27[:, KB9:2 * KB9], res1[:])
            res2 = sp.tile([128, KB9], f32)
            nc.vector.tensor_sub(res2[:], res1[:], CF27[:, KB9:2 * KB9])
            nc.vector.tensor_copy(CF27[:, 2 * KB9:KB], res2[:])

            # transpose [128,27] -> lhsT [27,128] via PE; PSUM->SBUF on ScalarE
            cfT_ps = pp.tile([KB, 128], bf16, tag="ps")
            nc.tensor.transpose(cfT_ps[:], CF27[:], ID[:])
            lhsT = cpool.tile([KB, 128], bf16)
            nc.scalar.copy(lhsT[:], cfT_ps[:])

            # --- smax: row-argmax in closed form, then 128-point reduce ---
            tq = mp.tile([128, RES], f32)   # continuous col index of row-argmax
            nc.vector.tensor_scalar(tq[:], dyj[:], kf[:], mi, AOp.mult, AOp.add)
            tqc = mp.tile([128, RES], f32)
            nc.vector.tensor_scalar(tqc[:], tq[:], 0.0, 127.0, AOp.max, AOp.min)
            tqr = mp.tile([128, RES], f32)  # round to nearest grid index
            nc.vector.tensor_scalar(tqr[:], tqc[:], MAGIC, MAGIC, AOp.add, AOp.subtract)
            dqi = mp.tile([128, RES], f32)  # i_q - mi
            nc.vector.tensor_scalar_sub(dqi[:], tqr[:], mi)
            w1 = mp.tile([128, RES], f32)
            nc.vector.tensor_scalar_mul(w1[:], dyj[:], Ci)
            w2 = mp.tile([128, RES], f32)
            nc.vector.scalar_tensor_tensor(w2[:], dqi[:], Bi, w1[:], AOp.mult, AOp.add)
            w3 = mp.tile([128, RES], f32)
            nc.vector.tensor_mul(w3[:], w2[:], dyj[:])
            w4 = mp.tile([128, RES], f32)
            nc.vector.scalar_tensor_tensor(w4[:], dqi[:], Ai, dqi[:], AOp.mult, AOp.mult)
            mrow = mp.tile([128, RES], f32)
            nc.vector.tensor_add(mrow[:], w3[:], w4[:])
            SMM = sp.tile([128, 2], f32)    # (smax, smin)
            nc.vector.tensor_reduce(SMM[:, 0:1], mrow[:], X, AOp.max)

            # --- smin: corner quadratic (s concave -> min at a grid corner)
            z1 = sp.tile([128, 4], f32)
            nc.vector.tensor_scalar_mul(z1[:], dyc[:], Ci)
            z2 = sp.tile([128, 4], f32)
            nc.vector.scalar_tensor_tensor(z2[:], dxc[:], Bi, z1[:], AOp.mult, AOp.add)
            z3 = sp.tile([128, 4], f32)
            nc.vector.tensor_mul(z3[:], z2[:], dyc[:])
            z4 = sp.tile([128, 4], f32)
            nc.vector.scalar_tensor_tensor(z4[:], dxc[:], Ai, dxc[:], AOp.mult, AOp.mult)
            zm = sp.tile([128, 4], f32)
            nc.vector.tensor_add(zm[:], z3[:], z4[:])
            nc.vector.tensor_reduce(SMM[:, 1:2], zm[:], X, AOp.min)

            # normalization scalars: R1 = 1/(e^smax - e^smin), r2 = e^smin * R1
            ESM = sp.tile([128, 2], f32)
            nc.scalar.activation(ESM[:], SMM[:], FT.Exp)
            Dn = sp.tile([128, 1], f32)
            nc.vector.tensor_sub(Dn[:], ESM[:, 0:1], ESM[:, 1:2])
            R1 = sp.tile([128, 1], f32)
            nc.vector.reciprocal(R1[:], Dn[:])
            r2 = sp.tile([128, 1], f32)
            nc.vector.tensor_mul(r2[:], ESM[:, 1:2], R1[:])

            # --- main loop: matmul -> exp -> normalize -> DMA out ---
            for ch in range(N_CHUNKS):
                ps = pp.tile([128, CHUNK], f32, tag="ps")
                for mm in range(MM_PER_CHUNK):
                    lo = ch * CHUNK + mm * MM_N
                    nc.tensor.matmul(
                        ps[:, mm * MM_N:(mm + 1) * MM_N],
                        lhsT[:],
                        BSt[:, lo:lo + MM_N],
                        start=True,
                        stop=True,
                    )
                e = iop.tile([128, CHUNK], f32, tag="e")
                nc.scalar.activation(e[:], ps[:], FT.Exp)
                o = iop.tile([128, CHUNK], f32, tag="o")
                nc.vector.tensor_scalar(o[:], e[:], R1[:], r2[:], AOp.mult, AOp.subtract)
                nc.sync.dma_start(out_ap[:, ch * CHUNK:(ch + 1) * CHUNK], o[:])

    nc.compile()
    return nc


def make_constants():
    idx = np.arange(RES, dtype=np.int64)
    i = np.tile(idx, RES)                        # flat idx n = j*RES + i
    j = np.repeat(idx, RES)
    rows9 = []
    for prod in (i * i, i * j, j * j):
        rows9.append(prod // 128)                # q < 128
        rows9.append(prod % 128)                 # r < 128
    rows9.append(i)
    rows9.append(j)
    rows9.append(np.ones(NPTS, dtype=np.int64))
    basis9 = np.stack(rows9).astype(np.float64)  # all small ints, exact in bf16
    basis = np.concatenate([basis9, basis9, basis9]).astype(ml_dtypes.bfloat16)
    fgrid = np.zeros((G_PER_CORE, RES + 8), np.float32)
    fgrid[:, 0:RES] = idx.astype(np.float32)[None, :]
    fgrid[:, RES:RES + 4] = np.array([0.0, 127.0, 0.0, 127.0], np.float32)
    fgrid[:, RES + 4:RES + 8] = np.array([0.0, 0.0, 127.0, 127.0], np.float32)
    ident = np.eye(128, dtype=ml_dtypes.bfloat16)
    return basis, fgrid, ident


def make_in_maps(mu, covar):
    mu = np.ascontiguousarray(np.asarray(mu), dtype=np.float32)
    covar = np.ascontiguousarray(np.asarray(covar), dtype=np.float32)
    G = mu.shape[0] * mu.shape[1]
    muf = mu.reshape(G, 2)
    cvf = covar.reshape(G, 4)
    basis, fgrid, ident = make_constants()
    in_maps = []
    for cid in range(N_CORES):
        sl = slice(cid * G_PER_CORE, (cid + 1) * G_PER_CORE)
        params = np.zeros((G_PER_CORE, 8), np.float32)
        params[:, 0] = muf[sl, 0]
        params[:, 1] = muf[sl, 1]
        params[:, 2] = cvf[sl, 0]   # a
        params[:, 3] = cvf[sl, 1]   # b
        params[:, 4] = cvf[sl, 2]   # c
        params[:, 5] = cvf[sl, 3]   # d
        params[:, 6] = muf[sl, 1]   # muy again (for the (mj, mi) affine)
        params[:, 7] = muf[sl, 0]   # mux again
        in_maps.append(
            {
                "params": params,
                "basis": basis,
                "fgrid": fgrid,
                "ident": ident,
            }
        )
    return in_maps


_NC_CACHE = None


def get_nc():
    global _NC_CACHE
    if _NC_CACHE is None:
        _NC_CACHE = build_nc()
    return _NC_CACHE


def kernel(mu, covar, _trace=False, _trace_kwargs=None):
    in_maps = make_in_maps(mu, covar)
    nc = get_nc()
    res = run_bass_kernel_spmd(
        nc, in_maps, core_ids=list(range(N_CORES)), trace=_trace,
        **(_trace_kwargs or {}),
    )
    outs = [np.asarray(res.results[i]["out"]) for i in range(N_CORES)]
    full = np.concatenate(outs, axis=0)           # [1024, 16384]
    out = full.reshape(16, 64, 1, RES, RES).astype(np.float32, copy=False)
    if _trace:
        return out, res
    return out
